# revision 15
# baseline (speedup 1.0000x reference)
"""Trainium2 Bass kernel for ModalityAwareDualAttention (dense_cnn).

Sharding: pure data-parallel over batch (32 -> 4 per core x 8 cores).
Per core: loop over P=3 parts; each part processes all BL=4 local batches
together (token-packed attention-value matmuls, batched SE gate).

Algebraic restructurings (exact up to fp assoc.):
  - depthwise scale/bias + 2x2-avg-pool 0.25 factor folded into Wq/Wk + biases
  - v computed transposed (vT = xd^T @ Wv^T), token-packed across the 4
    batches (384 tokens -> 3 full 128-wide PE column groups)
  - v-bias commutes through softmax (rows sum to 1); added at up-eviction
  - attention apply + bilinear 2x upsample + pa_gamma fused into two matmuls:
    up = vT^T @ (attn_n @ KT), KT = gamma * K_bilinear^T
  - s := xp + up_sb (pa_out) stored at eviction; its accum gives the SE
    global-avg-pool for free (1/384 folded into fc1 weights)
  - gates folded:  cw2 = mw + mw*ca_gamma*cw,  and since cw1 - cw2 = 1 - mw,
    final = cw2*s + (1-mw)*xp  -- one fused op per tile at blend time

Pipelining: vT is computed in four 512-column passes; the up/s/fc1 work for
the finished quarter is interleaved behind the next quarter's vT matmuls so
the PE never idles waiting for PSUM evictions.  The final blend of part p is
interleaved with part p+1's x loads/pooling/projections.
Matmuls and main streams bf16 (fp32 PSUM accumulation); x and out are
carried bf16 end-to-end (tolerance 2e-2 >> bf16 rounding).
"""

import numpy as np
import ml_dtypes

import concourse.bass as bass
import concourse.tile as tile
import concourse.mybir as mybir

F32 = mybir.dt.float32
BF16 = mybir.dt.bfloat16
AF = mybir.ActivationFunctionType
ALU = mybir.AluOpType

N_CORES = 8
B, C, H, W, P = 32, 2048, 48, 24, 3
BL = B // N_CORES          # 4 local batches per core
IC = 128                   # q/k inter channels
C4 = 512                   # SE bottleneck
PH = H // P                # 16
HD, WD = PH // 2, W // 2   # 8, 12
N = HD * WD                # 96 attention tokens per batch
HWP = PH * W               # 384 spatial positions per part
KC = C // 128              # 16 channel tiles
TOK = BL * N               # 384 packed tokens per part
NG = TOK // 128            # 3 token M-groups
NCH = 4                    # vT column passes
CHW = C // NCH             # 512 columns per pass

# smalls column map
QB, KB_, B1, VBG, B2, MWC, MW, MWM1 = 0, 1, 2, 6, 22, 38, 42, 46
NS = 50


def _up_matrix(n):
    """[2n, n] bilinear x2 upsample (align_corners=False, edge clamp)."""
    M = np.zeros((2 * n, n), np.float64)
    for o in range(2 * n):
        src = (o + 0.5) / 2.0 - 0.5
        i0 = int(np.floor(src))
        f = src - i0
        M[o, min(max(i0, 0), n - 1)] += 1.0 - f
        M[o, min(max(i0 + 1, 0), n - 1)] += f
    return M


def k_bilinear():
    """[384, 96] upsample matrix: flat(16,24) <- flat(8,12)."""
    return np.kron(_up_matrix(HD), _up_matrix(WD))


def split_excess_waits(nc, max_waits=1):
    """This walrus build rejects multi-sem-wait instructions on some opcodes;
    hoist extra waits onto preceding same-engine no-ops."""
    for f in nc.m.functions:
        for bb in f.blocks:
            insts = bb.instructions
            i = 0
            while i < len(insts):
                ins = insts[i]
                si = ins.sync_info
                if si is not None and si.on_wait and len(si.on_wait) > max_waits:
                    waits = list(si.on_wait)
                    extra, keep = waits[:-max_waits], waits[-max_waits:]
                    nops = []
                    for s in range(0, len(extra), max_waits):
                        nops.append(mybir.InstNoOp(
                            name=nc.get_next_instruction_name(),
                            engine=ins.engine, ins=[], outs=[],
                            sync_info=mybir.SyncInfo(
                                on_wait=extra[s:s + max_waits], on_update=[]),
                        ))
                    ins.sync_info = mybir.SyncInfo(
                        on_wait=keep, on_update=list(si.on_update or []))
                    insts[i:i] = nops
                    i += len(nops)
                i += 1


def build_program(split_waits=True):
    from contextlib import ExitStack
    nc = bass.Bass()

    x = nc.dram_tensor("x", [BL, C, H, W], BF16, kind="ExternalInput")
    wq = nc.dram_tensor("wq", [P, 128, KC * IC], BF16, kind="ExternalInput")
    wk = nc.dram_tensor("wk", [P, 128, KC * IC], BF16, kind="ExternalInput")
    wv = nc.dram_tensor("wv", [P, 128, KC, C], BF16, kind="ExternalInput")
    kt = nc.dram_tensor("kt", [P, N, HWP + 1], BF16, kind="ExternalInput")
    fc1 = nc.dram_tensor("fc1", [P, 4, 128, KC * 128], BF16,
                         kind="ExternalInput")
    fc2 = nc.dram_tensor("fc2", [P, 4, 128, 4 * C4], BF16,
                         kind="ExternalInput")
    sm = nc.dram_tensor("sm", [P, 128, NS], F32, kind="ExternalInput")
    vbgr = nc.dram_tensor("vbgr", [P, 1, C], BF16, kind="ExternalInput")
    out = nc.dram_tensor("out", [BL, C, H, W], BF16, kind="ExternalOutput")

    # x[b, (kc q), (p h), w] -> [b, q, kc, p, (h w)]
    xr = x.ap().rearrange("b (kc q) (p h) w -> b q kc p (h w)",
                          kc=KC, q=128, p=P)
    # out[b, (kg i q), (p h), w] -> [b, kg, q, i, p, (h w)]
    orr = out.ap().rearrange("b (kg i q) (p h) w -> b kg q i p (h w)",
                             kg=8, i=2, q=128, p=P)
    # wv[p, q, kc, c] pairs of kc per DMA
    wvr = wv.ap().rearrange("p q (k2 two) c -> p q k2 two c", two=2)

    with ExitStack() as ctx:
        tc = ctx.enter_context(tile.TileContext(nc))
        pool = lambda name, bufs, **kw: ctx.enter_context(
            tc.tile_pool(name=name, bufs=bufs, **kw))
        xp_pool = pool("xp", BL + 1)
        xd_pool = pool("xd", 1)
        t1_pool = pool("t1", 2)
        wq_pool = pool("wqp", 1)
        wk_pool = pool("wkp", 1)
        wv_pool = pool("wvp", 3)
        kt_pool = pool("ktp", 2)
        sm_pool = pool("smp", 2)
        fc1_pool = pool("fc1p", 4)
        fc2_pool = pool("fc2p", 2)
        qk_pool = pool("qk", 4)
        at_pool = pool("at", 3)
        ss_pool = pool("ss", 6)
        g_pool = pool("gg", 5)
        vt_pool = pool("vt", NG)
        vtb_pool = pool("vtb", BL)
        s_pool = pool("sp", BL)
        ga_pool = pool("ga", 2)
        xs_pool = pool("xs", 2)
        gap_pool = pool("gap", 2)
        h1_pool = pool("h1", 4)
        cws_pool = pool("cws", 2)
        cw_pool = pool("cw", 2)
        fin_pool = pool("fin", 2)
        tms_pool = pool("tms", 3)
        ps_sm = pool("ps_sm", 3, space="PSUM")
        ps_kw = pool("ps_kw", 1, space="PSUM")
        ps_vt = pool("ps_vt", NG, space="PSUM")
        ps_h = pool("ps_h", 1, space="PSUM")

        # deferred blend state from previous part
        prev = None
        kw_ps = ps_kw.tile([128, 128], F32, tag="kw", name="kw")

        def emit_blend_batch(st, b):
            """Blend batch b of a finished part: out = cw2*s + (1-mw)*xp."""
            p0, xp0, s0, cw0 = st
            for kg in range(8):
                fin = fin_pool.tile([128, 2, HWP], BF16, tag="fin",
                                    name="fin")
                for i in range(2):
                    kc = 2 * kg + i
                    if kc % 2 == 0:
                        nc.vector.scalar_tensor_tensor(
                            fin[:, i, :], s0[b][:, kc, :],
                            cw0[:, kc, b:b + 1], xp0[b][:, kc, :],
                            ALU.mult, ALU.add)
                    else:
                        tms = tms_pool.tile([128, HWP], BF16, tag="tms",
                                            name="tms")
                        nc.scalar.activation(tms[:], s0[b][:, kc, :],
                                             AF.Copy,
                                             scale=cw0[:, kc, b:b + 1])
                        nc.vector.tensor_tensor(fin[:, i, :], tms[:],
                                                xp0[b][:, kc, :], ALU.add)
                nc.gpsimd.dma_start(orr[b, kg, :, :, p0], fin[:])
                # keep-warm matmul paced by the blend stream (prevents the
                # HAM clock-gate from re-throttling the PE during
                # eviction/blend-only windows)
                nc.tensor.matmul(kw_ps[:, 0:16], s0[b][:, 0, 0:128],
                                 fin[:, 0, 0:16], start=True, stop=True)

        for p in range(P):
            # ---------- per-part weight DMAs ----------
            wq_t = wq_pool.tile([128, KC * IC], BF16, tag="wq", name="wq")
            nc.sync.dma_start(wq_t[:], wq.ap()[p])
            wk_t = wk_pool.tile([128, KC * IC], BF16, tag="wk", name="wk")
            nc.sync.dma_start(wk_t[:], wk.ap()[p])
            kt_t = kt_pool.tile([N, HWP + 1], BF16, tag="kt", name="kt")
            nc.sync.dma_start(kt_t[:], kt.ap()[p])
            sm_t = sm_pool.tile([128, NS], F32, tag="sm", name="sm")
            nc.sync.dma_start(sm_t[:], sm.ap()[p])

            # ---------- interleaved: blend(prev) | load+pool+qk(p) -------
            xp_t, qs_t, ks_t = [], [], []
            xd_t = xd_pool.tile([128, KC, TOK], BF16, tag="xd", name="xd")
            xs_t = xs_pool.tile([128, KC, BL], F32, tag="xs", name="xs")
            for b in range(BL):
                if prev is not None:
                    emit_blend_batch(prev, b)
                t = xp_pool.tile([128, KC, HWP], BF16, tag="xp", name="xp")
                nc.sync.dma_start(t[:], xr[b, :, :, p])
                xp_t.append(t)
                xv = t[:].rearrange("q kc (h w) -> q kc h w", h=PH)
                for k4 in range(0, KC, 4):
                    t1 = t1_pool.tile([128, 4, HD, W], BF16, tag="t1",
                                      name="t1")
                    nc.vector.tensor_tensor(
                        t1[:], xv[:, k4:k4 + 4, 0:PH:2, :],
                        xv[:, k4:k4 + 4, 1:PH:2, :], ALU.add)
                    t1v = t1[:].rearrange("q c h (w two) -> q c h w two",
                                          two=2)
                    nc.vector.tensor_tensor(
                        xd_t[:, k4:k4 + 4, b * N:(b + 1) * N].rearrange(
                            "q c (h w) -> q c h w", h=HD),
                        t1v[:, :, :, :, 0], t1v[:, :, :, :, 1], ALU.add)
                    nc.vector.tensor_reduce(
                        xs_t[:, k4:k4 + 4, b:b + 1],
                        xd_t[:, k4:k4 + 4, b * N:(b + 1) * N],
                        mybir.AxisListType.X, ALU.add)
                    nc.tensor.matmul(kw_ps[:, 16:32],
                                     t[:, 0, 0:128],
                                     xd_t[:, k4, b * N:b * N + 16],
                                     start=True, stop=True)
                # q/k per pair once both batches pooled
                if b % 2 == 1:
                    pr = b // 2
                    q_ps = ps_sm.tile([IC, 2 * N], F32, tag="ps", name="qps")
                    for kc in range(KC):
                        nc.tensor.matmul(
                            q_ps[:], wq_t[:, kc * IC:(kc + 1) * IC],
                            xd_t[:, kc, pr * 2 * N:(pr + 1) * 2 * N],
                            start=(kc == 0), stop=(kc == KC - 1))
                    qs = qk_pool.tile([IC, 2 * N], BF16, tag="qk", name="qs")
                    nc.scalar.activation(qs[:], q_ps[:], AF.Identity,
                                         bias=sm_t[:, QB:QB + 1])
                    qs_t.append(qs)
                    k_ps = ps_sm.tile([IC, 2 * N], F32, tag="ps", name="kps")
                    for kc in range(KC):
                        nc.tensor.matmul(
                            k_ps[:], wk_t[:, kc * IC:(kc + 1) * IC],
                            xd_t[:, kc, pr * 2 * N:(pr + 1) * 2 * N],
                            start=(kc == 0), stop=(kc == KC - 1))
                    ks = qk_pool.tile([IC, 2 * N], BF16, tag="qk", name="ks")
                    nc.scalar.activation(ks[:], k_ps[:], AF.Identity,
                                         bias=sm_t[:, KB_:KB_ + 1])
                    ks_t.append(ks)
            # fc1/fc2 streamed chunks (used only at SE time, loaded early)
            fc1_c = []
            for m in range(4):
                t = fc1_pool.tile([128, KC * 128], BF16, tag="fc1",
                                  name="fc1")
                nc.sync.dma_start(t[:], fc1.ap()[p, m])
                fc1_c.append(t)

            # ---------- vT in four 512-col passes; attention interleaved
            # into pass 0; up/s/fc1 for quarter q interleaved behind the
            # pass q+1 matmuls ----------
            vt_t = [vt_pool.tile([128, C], BF16, tag="vt", name="vt")
                    for _ in range(NG)]
            vtb_t = [vtb_pool.tile([N + 1, C], BF16, tag="vtb",
                                   name="vtb")
                     for _ in range(BL)]
            for j in range(BL):
                nc.gpsimd.dma_start(vtb_t[j][N:N + 1, :], vbgr.ap()[p])
            attn_n = [None] * BL
            g_sb = [None] * BL
            s_t = [s_pool.tile([128, KC, HWP], BF16, tag="sp", name="s")
                   for _ in range(BL)]
            ua_t = ga_pool.tile([128, KC, BL], F32, tag="ga", name="ua")
            gap_t = gap_pool.tile([128, KC, BL], BF16, tag="gap", name="gap")
            h_all = ps_h.tile([128, 16], F32, tag="psh", name="hall")

            def up_quarter(ch):
                """up matmuls + s eviction + fc1 for kc in quarter ch."""
                q0 = 4 * ch
                for kc in range(q0, q0 + 4):
                    for j in range(BL):
                        lhs = vtb_t[j][0:N + 1, kc * 128:(kc + 1) * 128]
                        up_ps = ps_sm.tile([128, HWP + 1], F32, tag="ps",
                                           name="ups")
                        nc.tensor.matmul(up_ps[:], lhs, g_sb[j][:],
                                         start=True, stop=True)
                        nc.vector.tensor_tensor(
                            s_t[j][:, kc, :], up_ps[:, 0:HWP],
                            xp_t[j][:, kc, :], ALU.add)
                        nc.scalar.copy(ua_t[:, kc, j:j + 1],
                                       up_ps[:, HWP:HWP + 1])
                nc.vector.tensor_tensor(gap_t[:, q0:q0 + 4, :],
                                        xs_t[:, q0:q0 + 4, :],
                                        ua_t[:, q0:q0 + 4, :], ALU.add)
                for j in range(BL):
                    nc.scalar.activation(
                        xp_t[j][:, q0:q0 + 4, :].rearrange(
                            "q c f -> q (c f)"),
                        xp_t[j][:, q0:q0 + 4, :].rearrange(
                            "q c f -> q (c f)"), AF.Copy,
                        scale=sm_t[:, MWM1 + j:MWM1 + j + 1])
                for kc in range(q0, q0 + 4):
                    for m in range(4):
                        nc.tensor.matmul(
                            h_all[:, m * 4:(m + 1) * 4],
                            fc1_c[m][:, kc * 128:(kc + 1) * 128],
                            gap_t[:, kc, :],
                            start=(kc == 0), stop=(kc == KC - 1))

            for ch in range(NCH):
                cl = ch * CHW
                vt_ps = [ps_vt.tile([128, CHW], F32, tag="psv", name="vps")
                         for _ in range(NG)]
                for k2 in range(KC // 2):
                    wv_t = wv_pool.tile([128, 2, CHW], BF16, tag="wv",
                                        name="wv")
                    nc.sync.dma_start(wv_t[:], wvr[p, :, k2, :, cl:cl + CHW])
                    for i2 in range(2):
                        kc = 2 * k2 + i2
                        for gi in range(NG):
                            nc.tensor.matmul(
                                vt_ps[gi][:],
                                xd_t[:, kc, gi * 128:(gi + 1) * 128],
                                wv_t[:, i2, :],
                                start=(kc == 0), stop=(kc == KC - 1))
                    # interleave attention (softmax without max-shift:
                    # |energy| ~ 1e-3, exp cannot overflow)
                    if ch == 0:
                        j, ph = k2 // 2, k2 % 2
                        if ph == 0:
                            pr, jo = j // 2, (j % 2) * N
                            e_ps = ps_sm.tile([N, N], F32, tag="ps",
                                              name="eps")
                            nc.tensor.matmul(
                                e_ps[:], qs_t[pr][:, jo:jo + N],
                                ks_t[pr][:, jo:jo + N],
                                start=True, stop=True)
                            a_e = at_pool.tile([N, N], BF16, tag="at",
                                               name="ae")
                            s_sum = ss_pool.tile([N, 1], F32, tag="ss",
                                                 name="ssum")
                            nc.scalar.activation(a_e[:], e_ps[:], AF.Exp,
                                                 accum_out=s_sum[:])
                            r_sum = ss_pool.tile([N, 1], F32, tag="ss",
                                                 name="rsum")
                            nc.vector.reciprocal(r_sum[:], s_sum[:])
                            a_n = at_pool.tile([N, N], BF16, tag="at",
                                               name="an")
                            nc.vector.tensor_scalar(a_n[:], a_e[:],
                                                    r_sum[:], None,
                                                    ALU.mult)
                            attn_n[j] = a_n
                        else:
                            g_ps = ps_sm.tile([N, HWP + 1], F32, tag="ps",
                                              name="gps")
                            nc.tensor.matmul(g_ps[:], attn_n[j][:],
                                             kt_t[:], start=True, stop=True)
                            gt = g_pool.tile([N + 1, HWP + 1], BF16,
                                             tag="g", name="g")
                            nc.scalar.activation(gt[0:N, :], g_ps[:],
                                                 AF.Copy)
                            nc.vector.memset(gt[N:N + 1, 0:HWP], 1.0)
                            nc.vector.memset(gt[N:N + 1, HWP:HWP + 1],
                                             float(HWP))
                            g_sb[j] = gt
                for gi in range(NG):
                    nc.scalar.activation(vt_t[gi][:, cl:cl + CHW],
                                         vt_ps[gi][:], AF.Copy)
                # token re-base: batch j's 96 token rows contiguous from 0
                cs = slice(cl, cl + CHW)
                nc.gpsimd.dma_start(vtb_t[0][0:96, cs], vt_t[0][0:96, cs])
                nc.gpsimd.dma_start(vtb_t[1][0:32, cs], vt_t[0][96:128, cs])
                nc.gpsimd.dma_start(vtb_t[1][32:96, cs], vt_t[1][0:64, cs])
                nc.gpsimd.dma_start(vtb_t[2][0:64, cs], vt_t[1][64:128, cs])
                nc.gpsimd.dma_start(vtb_t[2][64:96, cs], vt_t[2][0:32, cs])
                nc.gpsimd.dma_start(vtb_t[3][0:96, cs], vt_t[2][32:128, cs])
                up_quarter(ch)

            # ---------- SE gate tail ----------
            h1_t = []
            for m in range(4):
                hb = h1_pool.tile([128, BL], BF16, tag="h1", name="h1")
                nc.scalar.activation(hb[:], h_all[:, m * 4:(m + 1) * 4],
                                     AF.Relu, bias=sm_t[:, B1 + m:B1 + m + 1])
                h1_t.append(hb)
            fc2_c = []
            for kg in range(4):
                t = fc2_pool.tile([128, 4 * C4], BF16, tag="fc2",
                                  name="fc2")
                nc.sync.dma_start(t[:], fc2.ap()[p, kg])
                fc2_c.append(t)
            cw2_t = cw_pool.tile([128, KC, BL], F32, tag="cw", name="cw2")
            for kc in range(KC):
                c_ps = ps_sm.tile([128, BL], F32, tag="ps", name="cps")
                for m in range(4):
                    nc.tensor.matmul(
                        c_ps[:],
                        fc2_c[kc // 4][:, (kc % 4) * C4 + m * 128:
                                       (kc % 4) * C4 + (m + 1) * 128],
                        h1_t[m][:], start=(m == 0), stop=(m == 3))
                cwg = cws_pool.tile([128, BL], F32, tag="cws", name="cwg")
                nc.scalar.activation(cwg[:], c_ps[:], AF.Sigmoid,
                                     bias=sm_t[:, B2 + kc:B2 + kc + 1])
                # cw2 = mw + mwc*cw
                tmp = cws_pool.tile([128, BL], F32, tag="cws", name="tmp")
                nc.vector.tensor_tensor(tmp[:], cwg[:],
                                        sm_t[:, MWC:MWC + BL], ALU.mult)
                nc.vector.tensor_tensor(cw2_t[:, kc, :], tmp[:],
                                        sm_t[:, MW:MW + BL], ALU.add)

            prev = (p, xp_t, s_t, cw2_t)

        # final part's blend
        for b in range(BL):
            emit_blend_batch(prev, b)

    if split_waits:
        split_excess_waits(nc)
    return nc


# ---------------------------------------------------------------------------
# Host side
# ---------------------------------------------------------------------------

def _sigmoid(v):
    return 1.0 / (1.0 + np.exp(-v))


def _bf(a):
    return np.ascontiguousarray(a.astype(ml_dtypes.bfloat16))


def prepare_host_inputs(inputs):
    """Fold/transpose weights; returns per-core input dicts."""
    g = {k: np.asarray(v) for k, v in inputs.items()}
    x = np.asarray(g["x"], dtype=np.float32)

    # modality gate on host (tiny): mw [B, P]
    mf = g["modality"].astype(np.float64)[:, None]
    g1 = np.maximum(mf @ g["gate_w1"].astype(np.float64).T
                    + g["gate_b1"].astype(np.float64), 0.0)
    mw = _sigmoid(g1 @ g["gate_w2"].astype(np.float64).T
                  + g["gate_b2"].astype(np.float64))      # [B, P]

    paq = g["pa_q_w"].astype(np.float64)    # [P, IC, C]
    pak = g["pa_k_w"].astype(np.float64)
    pav = g["pa_v_w"].astype(np.float64)    # [P, C, C]
    dwq_w = g["pa_dw_q_w"].astype(np.float64)   # [P, C]
    dwq_b = g["pa_dw_q_b"].astype(np.float64)
    dwk_w = g["pa_dw_k_w"].astype(np.float64)
    dwk_b = g["pa_dw_k_b"].astype(np.float64)
    gam = g["pa_gamma"].astype(np.float64)      # [P]
    cgam = g["ca_gamma"].astype(np.float64)

    def chunkT(a, n128, inner):
        # [C_outer, inner] -> [128, n128 * inner], partition-major
        return a.reshape(n128, 128, inner).transpose(1, 0, 2).reshape(
            128, n128 * inner)

    wq_h = np.stack([chunkT((paq[pp] * dwq_w[pp][None, :] * 0.25).T, KC, IC)
                     for pp in range(P)])
    wk_h = np.stack([chunkT((pak[pp] * dwk_w[pp][None, :] * 0.25).T, KC, IC)
                     for pp in range(P)])
    qb_h = np.stack([g["pa_q_b"][pp] + paq[pp] @ dwq_b[pp] for pp in range(P)])
    kb_h = np.stack([g["pa_k_b"][pp] + pak[pp] @ dwk_b[pp] for pp in range(P)])
    wv_h = np.stack([chunkT(0.25 * pav[pp].T, KC, C).reshape(128, KC, C)
                     for pp in range(P)])
    vbg_h = np.stack([(gam[pp] * g["pa_v_b"][pp].astype(np.float64))
                      .reshape(KC, 128).T for pp in range(P)])  # [P,128,16]

    kb_mat = k_bilinear()                     # [384, 96]
    ktx = np.concatenate([kb_mat.T, kb_mat.sum(axis=0)[:, None]], axis=1)
    kt_h = np.stack([gam[pp] * ktx for pp in range(P)])         # [P,96,385]

    fc1w = g["ca_fc1_w"].astype(np.float64)    # [P, C4, C]
    fc2w = g["ca_fc2_w"].astype(np.float64)    # [P, C, C4]
    # fc1': (fc1_w/384).T [C, C4] -> [m, q(c chunk part.), kc, j] chunks
    fc1_h = np.stack([
        (fc1w[pp] / HWP).T.reshape(KC, 128, 4, 128)
        .transpose(2, 1, 0, 3).reshape(4, 128, KC * 128)
        for pp in range(P)])
    # fc2': fc2_w.T [C4, C] -> [kcg, q(c4 chunk part.), ci, m, j]
    fc2_h = np.stack([
        fc2w[pp].T.reshape(4, 128, 4, 4, 128)
        .transpose(2, 1, 3, 0, 4).reshape(4, 128, 4 * C4)
        for pp in range(P)])
    b1_h = np.stack([g["ca_fc1_b"][pp].reshape(4, 128).T for pp in range(P)])
    b2_h = np.stack([g["ca_fc2_b"][pp].reshape(KC, 128).T for pp in range(P)])

    vbgr_h = np.stack([(gam[pp] * g["pa_v_b"][pp].astype(np.float64))
                       [None, :] for pp in range(P)])   # [P,1,C]
    shared = {
        "wq": _bf(wq_h), "wk": _bf(wk_h), "wv": _bf(wv_h),
        "vbgr": _bf(vbgr_h),
        "kt": _bf(kt_h), "fc1": _bf(fc1_h.reshape(P, 4, 128, KC * 128)),
        "fc2": _bf(fc2_h.reshape(P, 4, 128, 4 * C4)),
    }
    xbf = x.astype(ml_dtypes.bfloat16)
    per_core = []
    for cc in range(N_CORES):
        mwl = mw[cc * BL:(cc + 1) * BL]       # [BL, P]
        sm_h = np.zeros((P, 128, NS), np.float32)
        for pp in range(P):
            sm_h[pp, :, QB] = qb_h[pp]
            sm_h[pp, :, KB_] = kb_h[pp]
            sm_h[pp, :, B1:B1 + 4] = b1_h[pp]
            sm_h[pp, :, VBG:VBG + KC] = vbg_h[pp]
            sm_h[pp, :, B2:B2 + KC] = b2_h[pp]
            sm_h[pp, :, MWC:MWC + BL] = (mwl[:, pp] * cgam[pp])[None, :]
            sm_h[pp, :, MW:MW + BL] = mwl[:, pp][None, :]
            sm_h[pp, :, MWM1:MWM1 + BL] = (1.0 - mwl[:, pp])[None, :]
        per_core.append({
            "x": np.ascontiguousarray(xbf[cc * BL:(cc + 1) * BL]),
            "sm": sm_h,
            **shared,
        })
    return per_core


_CACHE = {}


def kernel(**inputs):
    from concourse.bass_utils import run_bass_kernel_spmd

    per_core = prepare_host_inputs(inputs)
    if "nc" not in _CACHE:
        _CACHE["nc"] = build_program()
    nc = _CACHE["nc"]
    res = run_bass_kernel_spmd(nc, per_core, list(range(N_CORES)))
    outs = [res.results[c]["out"] for c in range(N_CORES)]
    return np.concatenate(outs, axis=0).astype(np.float32)


# revision 16
# speedup vs baseline: 1.0966x; 1.0966x over previous
"""Trainium2 Bass kernel for ModalityAwareDualAttention (dense_cnn).

Sharding: pure data-parallel over batch (32 -> 4 per core x 8 cores).
Per core: loop over P=3 parts; each part processes all BL=4 local batches
together (token-packed attention-value matmuls, batched SE gate).

Algebraic restructurings (exact up to fp assoc.):
  - depthwise scale/bias + 2x2-avg-pool 0.25 factor folded into Wq/Wk + biases
  - v computed transposed (vT = xd^T @ Wv^T), token-packed across the 4
    batches (384 tokens -> 3 full 128-wide PE column groups)
  - gamma*v_bias injected via an extra token row (vT row 96 = gamma*vb,
    G row 96 = 1); attention apply + bilinear 2x upsample + pa_gamma fused:
    up = vT^T @ (attn_n @ KT), KT = gamma * K_bilinear^T (+ col-sum column)
  - the SE global-avg-pool of `up` comes from N=1 matmuls against KT's
    column-sum column BEFORE up itself is computed -- the SE gate is ready
    early, so `up` is blended straight out of PSUM in one fused op:
      final = cw1*xp + cw2*(up + gamma*vb)
      cw2 = mw + mw*ca_gamma*cw ,  cw1 = cw2 + (1 - mw)
    (xp is pre-scaled by cw1 in place; 1/384 folded into fc1 weights)
  - the up+blend of part p is deferred and interleaved into part p+1's
    x-load/pooling prologue so the PE never drains between parts
Matmuls and main streams bf16 (fp32 PSUM accumulation); x and out are
carried bf16 end-to-end (tolerance 2e-2 >> bf16 rounding).
"""

import numpy as np
import ml_dtypes

import concourse.bass as bass
import concourse.tile as tile
import concourse.mybir as mybir

F32 = mybir.dt.float32
BF16 = mybir.dt.bfloat16
AF = mybir.ActivationFunctionType
ALU = mybir.AluOpType

N_CORES = 8
B, C, H, W, P = 32, 2048, 48, 24, 3
BL = B // N_CORES          # 4 local batches per core
IC = 128                   # q/k inter channels
C4 = 512                   # SE bottleneck
PH = H // P                # 16
HD, WD = PH // 2, W // 2   # 8, 12
N = HD * WD                # 96 attention tokens per batch
HWP = PH * W               # 384 spatial positions per part
KC = C // 128              # 16 channel tiles
TOK = BL * N               # 384 packed tokens per part
NG = TOK // 128            # 3 token M-groups
NCH = 4                    # vT column passes
CHW = C // NCH             # 512 columns per pass

# smalls column map
QB, KB_, B1, B2, MWC, MW = 0, 1, 2, 6, 22, 26
NS = 30


def _up_matrix(n):
    """[2n, n] bilinear x2 upsample (align_corners=False, edge clamp)."""
    M = np.zeros((2 * n, n), np.float64)
    for o in range(2 * n):
        src = (o + 0.5) / 2.0 - 0.5
        i0 = int(np.floor(src))
        f = src - i0
        M[o, min(max(i0, 0), n - 1)] += 1.0 - f
        M[o, min(max(i0 + 1, 0), n - 1)] += f
    return M


def k_bilinear():
    """[384, 96] upsample matrix: flat(16,24) <- flat(8,12)."""
    return np.kron(_up_matrix(HD), _up_matrix(WD))


def split_excess_waits(nc, max_waits=1):
    """This walrus build rejects multi-sem-wait instructions on some opcodes;
    hoist extra waits onto preceding same-engine no-ops."""
    for f in nc.m.functions:
        for bb in f.blocks:
            insts = bb.instructions
            i = 0
            while i < len(insts):
                ins = insts[i]
                si = ins.sync_info
                if si is not None and si.on_wait and len(si.on_wait) > max_waits:
                    waits = list(si.on_wait)
                    extra, keep = waits[:-max_waits], waits[-max_waits:]
                    nops = []
                    for s in range(0, len(extra), max_waits):
                        nops.append(mybir.InstNoOp(
                            name=nc.get_next_instruction_name(),
                            engine=ins.engine, ins=[], outs=[],
                            sync_info=mybir.SyncInfo(
                                on_wait=extra[s:s + max_waits], on_update=[]),
                        ))
                    ins.sync_info = mybir.SyncInfo(
                        on_wait=keep, on_update=list(si.on_update or []))
                    insts[i:i] = nops
                    i += len(nops)
                i += 1


def build_program(split_waits=True):
    from contextlib import ExitStack
    nc = bass.Bass()

    x = nc.dram_tensor("x", [BL, C, H, W], BF16, kind="ExternalInput")
    wq = nc.dram_tensor("wq", [P, 128, KC * IC], BF16, kind="ExternalInput")
    wk = nc.dram_tensor("wk", [P, 128, KC * IC], BF16, kind="ExternalInput")
    wv = nc.dram_tensor("wv", [P, 128, KC, C], BF16, kind="ExternalInput")
    kt = nc.dram_tensor("kt", [P, N, HWP + 1], BF16, kind="ExternalInput")
    fc1 = nc.dram_tensor("fc1", [P, 4, 128, KC * 128], BF16,
                         kind="ExternalInput")
    fc2 = nc.dram_tensor("fc2", [P, 4, 128, 4 * C4], BF16,
                         kind="ExternalInput")
    sm = nc.dram_tensor("sm", [P, 128, NS], F32, kind="ExternalInput")
    vbgr = nc.dram_tensor("vbgr", [P, 1, C], BF16, kind="ExternalInput")
    out = nc.dram_tensor("out", [BL, C, H, W], BF16, kind="ExternalOutput")

    # x[b, (kc q), (p h), w] -> [b, q, kc, p, (h w)]
    xr = x.ap().rearrange("b (kc q) (p h) w -> b q kc p (h w)",
                          kc=KC, q=128, p=P)
    # out[b, (kg i q), (p h), w] -> [b, kg, q, i, p, (h w)]
    orr = out.ap().rearrange("b (kg i q) (p h) w -> b kg q i p (h w)",
                             kg=8, i=2, q=128, p=P)
    # wv[p, q, kc, c] pairs of kc per DMA
    wvr = wv.ap().rearrange("p q (k2 two) c -> p q k2 two c", two=2)

    with ExitStack() as ctx:
        tc = ctx.enter_context(tile.TileContext(nc))
        pool = lambda name, bufs, **kw: ctx.enter_context(
            tc.tile_pool(name=name, bufs=bufs, **kw))
        xp_pool = pool("xp", 2 * BL - 1)
        xd_pool = pool("xd", 1)
        t1_pool = pool("t1", 2)
        wq_pool = pool("wqp", 1)
        wk_pool = pool("wkp", 1)
        wv_pool = pool("wvp", 3)
        kt_pool = pool("ktp", 2)
        sm_pool = pool("smp", 2)
        fc1_pool = pool("fc1p", 4)
        fc2_pool = pool("fc2p", 2)
        qk_pool = pool("qk", 4)
        at_pool = pool("at", 3)
        ss_pool = pool("ss", 6)
        g_pool = pool("gg", 2 * BL)
        vt_pool = pool("vt", NG)
        vtb_pool = pool("vtb", 2 * BL)
        xs_pool = pool("xs", 2)
        gap_pool = pool("gap", 2)
        h1_pool = pool("h1", 4)
        cws_pool = pool("cws", 2)
        cw_pool = pool("cw", 4)
        fin_pool = pool("fin", 3)
        ps_sm = pool("ps_sm", 4, space="PSUM")
        ps_vt = pool("ps_vt", NG, space="PSUM")
        ps_ua = pool("ps_ua", 1, space="PSUM")

        # the up+blend of part p is emitted inside part p+1's prologue
        prev = None

        def emit_upblend(st, kgs):
            """up matmuls + PSUM blend + store for 2-kc groups kgs of a
            finished part: out = cw2*(up+gvb) + cw1*xp (xp pre-scaled)."""
            p0, xp0, vtb0, g0, cw2 = st
            for kg in kgs:
                for j in range(BL):
                    fin = fin_pool.tile([128, 2, HWP], BF16, tag="fin",
                                        name="fin")
                    for i in range(2):
                        kc = 2 * kg + i
                        up_ps = ps_sm.tile([128, HWP + 1], F32, tag="ps",
                                           name="ups")
                        nc.tensor.matmul(
                            up_ps[:],
                            vtb0[j][0:N + 1, kc * 128:(kc + 1) * 128],
                            g0[j][:], start=True, stop=True)
                        nc.vector.scalar_tensor_tensor(
                            fin[:, i, :], up_ps[:, 0:HWP],
                            cw2[:, kc, j:j + 1], xp0[j][:, kc, :],
                            ALU.mult, ALU.add)
                    nc.gpsimd.dma_start(orr[j, kg, :, :, p0], fin[:])

        for p in range(P):
            # ---------- per-part weight DMAs ----------
            wq_t = wq_pool.tile([128, KC * IC], BF16, tag="wq", name="wq")
            nc.sync.dma_start(wq_t[:], wq.ap()[p])
            wk_t = wk_pool.tile([128, KC * IC], BF16, tag="wk", name="wk")
            nc.sync.dma_start(wk_t[:], wk.ap()[p])
            kt_t = kt_pool.tile([N, HWP + 1], BF16, tag="kt", name="kt")
            nc.sync.dma_start(kt_t[:], kt.ap()[p])
            sm_t = sm_pool.tile([128, NS], F32, tag="sm", name="sm")
            nc.sync.dma_start(sm_t[:], sm.ap()[p])

            # ---------- x loads + pooling + q/k, interleaved with the
            # deferred up+blend of the previous part ----------
            xp_t, qs_t, ks_t = [], [], []
            xd_t = xd_pool.tile([128, KC, TOK], BF16, tag="xd", name="xd")
            xs_t = xs_pool.tile([128, KC, BL], F32, tag="xs", name="xs")
            for b in range(BL):
                t = xp_pool.tile([128, KC, HWP], BF16, tag="xp", name="xp")
                nc.sync.dma_start(t[:], xr[b, :, :, p])
                xp_t.append(t)
                if prev is not None:
                    emit_upblend(prev, range(2 * b, 2 * b + 2))
                xv = t[:].rearrange("q kc (h w) -> q kc h w", h=PH)
                for k4 in range(0, KC, 4):
                    t1 = t1_pool.tile([128, 4, HD, W], BF16, tag="t1",
                                      name="t1")
                    nc.vector.tensor_tensor(
                        t1[:], xv[:, k4:k4 + 4, 0:PH:2, :],
                        xv[:, k4:k4 + 4, 1:PH:2, :], ALU.add)
                    t1v = t1[:].rearrange("q c h (w two) -> q c h w two",
                                          two=2)
                    nc.vector.tensor_tensor(
                        xd_t[:, k4:k4 + 4, b * N:(b + 1) * N].rearrange(
                            "q c (h w) -> q c h w", h=HD),
                        t1v[:, :, :, :, 0], t1v[:, :, :, :, 1], ALU.add)
                    nc.vector.tensor_reduce(
                        xs_t[:, k4:k4 + 4, b:b + 1],
                        xd_t[:, k4:k4 + 4, b * N:(b + 1) * N],
                        mybir.AxisListType.X, ALU.add)
                # q/k per pair once both batches pooled
                if b % 2 == 1:
                    pr = b // 2
                    q_ps = ps_sm.tile([IC, 2 * N], F32, tag="ps", name="qps")
                    for kc in range(KC):
                        nc.tensor.matmul(
                            q_ps[:], wq_t[:, kc * IC:(kc + 1) * IC],
                            xd_t[:, kc, pr * 2 * N:(pr + 1) * 2 * N],
                            start=(kc == 0), stop=(kc == KC - 1))
                    qs = qk_pool.tile([IC, 2 * N], BF16, tag="qk", name="qs")
                    nc.scalar.activation(qs[:], q_ps[:], AF.Identity,
                                         bias=sm_t[:, QB:QB + 1])
                    qs_t.append(qs)
                    k_ps = ps_sm.tile([IC, 2 * N], F32, tag="ps", name="kps")
                    for kc in range(KC):
                        nc.tensor.matmul(
                            k_ps[:], wk_t[:, kc * IC:(kc + 1) * IC],
                            xd_t[:, kc, pr * 2 * N:(pr + 1) * 2 * N],
                            start=(kc == 0), stop=(kc == KC - 1))
                    ks = qk_pool.tile([IC, 2 * N], BF16, tag="qk", name="ks")
                    nc.scalar.activation(ks[:], k_ps[:], AF.Identity,
                                         bias=sm_t[:, KB_:KB_ + 1])
                    ks_t.append(ks)
            prev = None

            # fc1 streamed chunks (used at SE time, loaded early)
            fc1_c = []
            for m in range(4):
                t = fc1_pool.tile([128, KC * 128], BF16, tag="fc1",
                                  name="fc1")
                nc.sync.dma_start(t[:], fc1.ap()[p, m])
                fc1_c.append(t)

            # ---------- vT in four 512-col passes; attention interleaved
            # into pass 0 ----------
            vt_t = [vt_pool.tile([128, C], BF16, tag="vt", name="vt")
                    for _ in range(NG)]
            vtb_t = [vtb_pool.tile([N + 1, C], BF16, tag="vtb", name="vtb")
                     for _ in range(BL)]
            for j in range(BL):
                nc.gpsimd.dma_start(vtb_t[j][N:N + 1, :], vbgr.ap()[p])
            attn_n = [None] * BL
            g_sb = [None] * BL
            for ch in range(NCH):
                cl = ch * CHW
                vt_ps = [ps_vt.tile([128, CHW], F32, tag="psv", name="vps")
                         for _ in range(NG)]
                for k2 in range(KC // 2):
                    wv_t = wv_pool.tile([128, 2, CHW], BF16, tag="wv",
                                        name="wv")
                    nc.sync.dma_start(wv_t[:], wvr[p, :, k2, :, cl:cl + CHW])
                    for i2 in range(2):
                        kc = 2 * k2 + i2
                        for gi in range(NG):
                            nc.tensor.matmul(
                                vt_ps[gi][:],
                                xd_t[:, kc, gi * 128:(gi + 1) * 128],
                                wv_t[:, i2, :],
                                start=(kc == 0), stop=(kc == KC - 1))
                    # interleave attention (softmax without max-shift:
                    # |energy| ~ 1e-3, exp cannot overflow)
                    if ch == 0:
                        j, ph = k2 // 2, k2 % 2
                        if ph == 0:
                            pr, jo = j // 2, (j % 2) * N
                            e_ps = ps_sm.tile([N, N], F32, tag="ps",
                                              name="eps")
                            nc.tensor.matmul(
                                e_ps[:], qs_t[pr][:, jo:jo + N],
                                ks_t[pr][:, jo:jo + N],
                                start=True, stop=True)
                            a_e = at_pool.tile([N, N], BF16, tag="at",
                                               name="ae")
                            s_sum = ss_pool.tile([N, 1], F32, tag="ss",
                                                 name="ssum")
                            nc.scalar.activation(a_e[:], e_ps[:], AF.Exp,
                                                 accum_out=s_sum[:])
                            r_sum = ss_pool.tile([N, 1], F32, tag="ss",
                                                 name="rsum")
                            nc.vector.reciprocal(r_sum[:], s_sum[:])
                            a_n = at_pool.tile([N, N], BF16, tag="at",
                                               name="an")
                            nc.vector.tensor_scalar(a_n[:], a_e[:],
                                                    r_sum[:], None,
                                                    ALU.mult)
                            attn_n[j] = a_n
                        else:
                            g_ps = ps_sm.tile([N, HWP + 1], F32, tag="ps",
                                              name="gps")
                            nc.tensor.matmul(g_ps[:], attn_n[j][:],
                                             kt_t[:], start=True, stop=True)
                            gt = g_pool.tile([N + 1, HWP + 1], BF16,
                                             tag="g", name="g")
                            nc.scalar.activation(gt[0:N, :], g_ps[:],
                                                 AF.Copy)
                            nc.vector.memset(gt[N:N + 1, 0:HWP], 1.0)
                            nc.vector.memset(gt[N:N + 1, HWP:HWP + 1],
                                             float(HWP))
                            g_sb[j] = gt
                for gi in range(NG):
                    nc.scalar.activation(vt_t[gi][:, cl:cl + CHW],
                                         vt_ps[gi][:], AF.Copy)
                # token re-base: batch j's 96 token rows contiguous from 0
                cs = slice(cl, cl + CHW)
                nc.gpsimd.dma_start(vtb_t[0][0:96, cs], vt_t[0][0:96, cs])
                nc.gpsimd.dma_start(vtb_t[1][0:32, cs], vt_t[0][96:128, cs])
                nc.gpsimd.dma_start(vtb_t[1][32:96, cs], vt_t[1][0:64, cs])
                nc.gpsimd.dma_start(vtb_t[2][0:64, cs], vt_t[1][64:128, cs])
                nc.gpsimd.dma_start(vtb_t[2][64:96, cs], vt_t[2][0:32, cs])
                nc.gpsimd.dma_start(vtb_t[3][0:96, cs], vt_t[2][32:128, cs])

            # ---------- SE gate: mean(up) via N=1 matmuls against the
            # column-sum column of KT, before up itself exists ----------
            ua_ps = ps_ua.tile([128, KC, BL], F32, tag="ua", name="ua")
            for kc in range(KC):
                for j in range(BL):
                    nc.tensor.matmul(
                        ua_ps[:, kc, j:j + 1],
                        vtb_t[j][0:N + 1, kc * 128:(kc + 1) * 128],
                        g_sb[j][0:N + 1, HWP:HWP + 1],
                        start=True, stop=True)
            gap_t = gap_pool.tile([128, KC, BL], BF16, tag="gap", name="gap")
            nc.vector.tensor_tensor(gap_t[:], xs_t[:], ua_ps[:], ALU.add)
            h1_t = []
            for m in range(4):
                h_ps = ps_sm.tile([128, BL], F32, tag="ps", name="hps")
                for kc in range(KC):
                    nc.tensor.matmul(
                        h_ps[:],
                        fc1_c[m][:, kc * 128:(kc + 1) * 128],
                        gap_t[:, kc, :],
                        start=(kc == 0), stop=(kc == KC - 1))
                hb = h1_pool.tile([128, BL], BF16, tag="h1", name="h1")
                nc.scalar.activation(hb[:], h_ps[:], AF.Relu,
                                     bias=sm_t[:, B1 + m:B1 + m + 1])
                h1_t.append(hb)
            fc2_c = []
            for kg in range(4):
                t = fc2_pool.tile([128, 4 * C4], BF16, tag="fc2",
                                  name="fc2")
                nc.sync.dma_start(t[:], fc2.ap()[p, kg])
                fc2_c.append(t)
            cw1_t = cw_pool.tile([128, KC, BL], F32, tag="cw", name="cw1")
            cw2_t = cw_pool.tile([128, KC, BL], F32, tag="cw", name="cw2")
            for kc in range(KC):
                c_ps = ps_sm.tile([128, BL], F32, tag="ps", name="cps")
                for m in range(4):
                    nc.tensor.matmul(
                        c_ps[:],
                        fc2_c[kc // 4][:, (kc % 4) * C4 + m * 128:
                                       (kc % 4) * C4 + (m + 1) * 128],
                        h1_t[m][:], start=(m == 0), stop=(m == 3))
                cwg = cws_pool.tile([128, BL], F32, tag="cws", name="cwg")
                nc.scalar.activation(cwg[:], c_ps[:], AF.Sigmoid,
                                     bias=sm_t[:, B2 + kc:B2 + kc + 1])
                # tmp = mwc*cw; cw2 = mw + tmp; cw1 = 1 + tmp
                tmp = cws_pool.tile([128, BL], F32, tag="cws", name="tmp")
                nc.vector.tensor_tensor(tmp[:], cwg[:],
                                        sm_t[:, MWC:MWC + BL], ALU.mult)
                nc.vector.tensor_tensor(cw2_t[:, kc, :], tmp[:],
                                        sm_t[:, MW:MW + BL], ALU.add)
                nc.vector.tensor_scalar(cw1_t[:, kc, :], tmp[:], 1.0, None,
                                        ALU.add)

            # xp scaled in place by cw1 (per-channel) for the PSUM blend
            for j in range(BL):
                for kc in range(KC):
                    nc.scalar.activation(
                        xp_t[j][:, kc, :], xp_t[j][:, kc, :], AF.Copy,
                        scale=cw1_t[:, kc, j:j + 1])

            prev = (p, xp_t, vtb_t, g_sb, cw2_t)

        # final part's up+blend
        emit_upblend(prev, range(8))

    if split_waits:
        split_excess_waits(nc)
    return nc


# ---------------------------------------------------------------------------
# Host side
# ---------------------------------------------------------------------------

def _sigmoid(v):
    return 1.0 / (1.0 + np.exp(-v))


def _bf(a):
    return np.ascontiguousarray(a.astype(ml_dtypes.bfloat16))


def prepare_host_inputs(inputs):
    """Fold/transpose weights; returns per-core input dicts."""
    g = {k: np.asarray(v) for k, v in inputs.items()}
    x = np.asarray(g["x"], dtype=np.float32)

    # modality gate on host (tiny): mw [B, P]
    mf = g["modality"].astype(np.float64)[:, None]
    g1 = np.maximum(mf @ g["gate_w1"].astype(np.float64).T
                    + g["gate_b1"].astype(np.float64), 0.0)
    mw = _sigmoid(g1 @ g["gate_w2"].astype(np.float64).T
                  + g["gate_b2"].astype(np.float64))      # [B, P]

    paq = g["pa_q_w"].astype(np.float64)    # [P, IC, C]
    pak = g["pa_k_w"].astype(np.float64)
    pav = g["pa_v_w"].astype(np.float64)    # [P, C, C]
    dwq_w = g["pa_dw_q_w"].astype(np.float64)   # [P, C]
    dwq_b = g["pa_dw_q_b"].astype(np.float64)
    dwk_w = g["pa_dw_k_w"].astype(np.float64)
    dwk_b = g["pa_dw_k_b"].astype(np.float64)
    gam = g["pa_gamma"].astype(np.float64)      # [P]
    cgam = g["ca_gamma"].astype(np.float64)

    def chunkT(a, n128, inner):
        # [C_outer, inner] -> [128, n128 * inner], partition-major
        return a.reshape(n128, 128, inner).transpose(1, 0, 2).reshape(
            128, n128 * inner)

    wq_h = np.stack([chunkT((paq[pp] * dwq_w[pp][None, :] * 0.25).T, KC, IC)
                     for pp in range(P)])
    wk_h = np.stack([chunkT((pak[pp] * dwk_w[pp][None, :] * 0.25).T, KC, IC)
                     for pp in range(P)])
    qb_h = np.stack([g["pa_q_b"][pp] + paq[pp] @ dwq_b[pp] for pp in range(P)])
    kb_h = np.stack([g["pa_k_b"][pp] + pak[pp] @ dwk_b[pp] for pp in range(P)])
    wv_h = np.stack([chunkT(0.25 * pav[pp].T, KC, C).reshape(128, KC, C)
                     for pp in range(P)])

    kb_mat = k_bilinear()                     # [384, 96]
    ktx = np.concatenate([kb_mat.T, kb_mat.sum(axis=0)[:, None]], axis=1)
    kt_h = np.stack([gam[pp] * ktx for pp in range(P)])         # [P,96,385]

    fc1w = g["ca_fc1_w"].astype(np.float64)    # [P, C4, C]
    fc2w = g["ca_fc2_w"].astype(np.float64)    # [P, C, C4]
    # fc1': (fc1_w/384).T [C, C4] -> [m, q(c chunk part.), kc, j] chunks
    fc1_h = np.stack([
        (fc1w[pp] / HWP).T.reshape(KC, 128, 4, 128)
        .transpose(2, 1, 0, 3).reshape(4, 128, KC * 128)
        for pp in range(P)])
    # fc2': fc2_w.T [C4, C] -> [kcg, q(c4 chunk part.), ci, m, j]
    fc2_h = np.stack([
        fc2w[pp].T.reshape(4, 128, 4, 4, 128)
        .transpose(2, 1, 3, 0, 4).reshape(4, 128, 4 * C4)
        for pp in range(P)])
    b1_h = np.stack([g["ca_fc1_b"][pp].reshape(4, 128).T for pp in range(P)])
    b2_h = np.stack([g["ca_fc2_b"][pp].reshape(KC, 128).T for pp in range(P)])

    vbgr_h = np.stack([(gam[pp] * g["pa_v_b"][pp].astype(np.float64))
                       [None, :] for pp in range(P)])   # [P,1,C]
    shared = {
        "wq": _bf(wq_h), "wk": _bf(wk_h), "wv": _bf(wv_h),
        "vbgr": _bf(vbgr_h),
        "kt": _bf(kt_h), "fc1": _bf(fc1_h.reshape(P, 4, 128, KC * 128)),
        "fc2": _bf(fc2_h.reshape(P, 4, 128, 4 * C4)),
    }
    xbf = x.astype(ml_dtypes.bfloat16)
    per_core = []
    for cc in range(N_CORES):
        mwl = mw[cc * BL:(cc + 1) * BL]       # [BL, P]
        sm_h = np.zeros((P, 128, NS), np.float32)
        for pp in range(P):
            sm_h[pp, :, QB] = qb_h[pp]
            sm_h[pp, :, KB_] = kb_h[pp]
            sm_h[pp, :, B1:B1 + 4] = b1_h[pp]
            sm_h[pp, :, B2:B2 + KC] = b2_h[pp]
            sm_h[pp, :, MWC:MWC + BL] = (mwl[:, pp] * cgam[pp])[None, :]
            sm_h[pp, :, MW:MW + BL] = mwl[:, pp][None, :]
        per_core.append({
            "x": np.ascontiguousarray(xbf[cc * BL:(cc + 1) * BL]),
            "sm": sm_h,
            **shared,
        })
    return per_core


_CACHE = {}


def kernel(**inputs):
    from concourse.bass_utils import run_bass_kernel_spmd

    per_core = prepare_host_inputs(inputs)
    if "nc" not in _CACHE:
        _CACHE["nc"] = build_program()
    nc = _CACHE["nc"]
    res = run_bass_kernel_spmd(nc, per_core, list(range(N_CORES)))
    outs = [res.results[c]["out"] for c in range(N_CORES)]
    return np.concatenate(outs, axis=0).astype(np.float32)
